# revision 15
# baseline (speedup 1.0000x reference)
"""BatchHardTripletLoss on 8 trn2 NeuronCores (Bass/Tile, SPMD data-parallel).

Label-sorted fp8 design, v3:

Host stable-sorts rows by label, L2-normalizes, and quantizes to fp8 e4m3.
After sorting, all same-label pairs of an anchor lie within +-W columns of
its own position (W >= max class size - 1).  Each core c anchors sorted rows
[512c, 512c+512) and sees all 4096 columns in circular order starting at
512c - W, so its first 512+2W columns (the "own+halo" window) contain every
same-label column of its anchors.  Consequences:

  * the label mask (-4 on same-label pairs) only touches the own+halo
    window; it is applied on the Vector engine as a fused psum+mask ->
    fp16 copy (host ships a [512, 512+2W] fp8 additive mask), keeping the
    Tensor engine free for the Gram;
  * hardest-positive (min) only scans a 128+2W band of the masked window
    per 128-anchor tile;
  * the 7 "far" chunks need only an unmasked max (hardest negative).

Main Gram runs as fp8 DoubleRow matmuls (K=256 per instruction).  Max
pipeline: ACT copies far psum chunks -> fp16 SBUF, DVE folds them into a
running elementwise-max chain, one final free-dim reduce per anchor tile.
The ragged chunks (halo tail, last far chunk) reduce straight from PSUM on
DVE and are processed early so no reduction backlog trails the last matmul.
The valid-anchor mask is applied via the final partition-sum matmul
(lhsT = valid column, rhs = per-anchor relu losses, bf16).

per-anchor loss = relu((max - min) + margin - 4); host adds the 8*4
partials and divides by n_valid.

Numerics: fp8 e4m3 Gram + fp16 staging measures rel err ~3e-4 vs the f32
reference on the fixed problem instance (tolerance 2e-2).
"""

import os
from contextlib import ExitStack

import numpy as np
import ml_dtypes

import concourse.bass as bass
import concourse.bacc as bacc
import concourse.mybir as mybir
import concourse.tile as tile
from concourse.bass_utils import run_bass_kernel_spmd

F32 = mybir.dt.float32
FP16 = mybir.dt.float16
BF16 = mybir.dt.bfloat16
FP8 = mybir.dt.float8e4
AF = mybir.ActivationFunctionType
ALU = mybir.AluOpType
AX = mybir.AxisListType
DR = mybir.MatmulPerfMode.DoubleRow

B, D, C = 4096, 512, 512
NCORES = 8
RPC = B // NCORES            # anchors per core = 512
NM = RPC // 128              # anchor tiles per core = 4
KD = D // 128                # k-subtiles = 4
MARGIN = 0.2
BIG = 4.0


def build_program(W):
    H2 = 2 * W
    OWNW = 512 + H2          # own+halo window width
    # column layout (host-chosen): [own+halo | ragged far | full far chunks]
    nfull = (B - OWNW) // 512
    ragw = B - OWNW - nfull * 512
    # processing order: own512, halo tail, ragged far chunk, then full fars
    order = [("own", 0, 512), ("ownh", 512, H2), ("rag", OWNW, ragw)]
    order += [("far", OWNW + ragw + 512 * i, 512) for i in range(nfull)]
    assert 512 + ragw + H2 == 1024 and nfull % 2 == 0, (ragw, H2, nfull)

    nc = bacc.Bacc("TRN2", target_bir_lowering=False, debug=False)
    ET_d = nc.declare_dram_parameter("ET", [D, B], FP8, isOutput=False)
    MSK_d = nc.declare_dram_parameter("MSK", [RPC, OWNW], FP8, isOutput=False)
    val_d = nc.declare_dram_parameter("valid", [128, NM], BF16, isOutput=False)
    out_d = nc.declare_dram_parameter("out", [1, NM], F32, isOutput=True)

    with tile.TileContext(nc) as tc, ExitStack() as ctx:
        const = ctx.enter_context(tc.tile_pool(name="const", bufs=1))
        bigp = ctx.enter_context(tc.tile_pool(name="bigp", bufs=1))
        fstg = ctx.enter_context(tc.tile_pool(name="fstg", bufs=1))
        smalls = ctx.enter_context(tc.tile_pool(name="small", bufs=1))
        psM = ctx.enter_context(tc.tile_pool(name="psM", bufs=6, space="PSUM"))
        psA = ctx.enter_context(tc.tile_pool(name="psA", bufs=1, space="PSUM"))
        psO = ctx.enter_context(tc.tile_pool(name="psO", bufs=1, space="PSUM"))

        bconst = const.tile([128, 1], F32, tag="bconst")
        nc.vector.memset(bconst[:], MARGIN - BIG)

        # ---- input DMA ------------------------------------------------------
        val_t = const.tile([128, NM], BF16, tag="val")
        et = bigp.tile([128, KD, B], FP8, tag="et")
        msk = [
            smalls.tile([128, OWNW], FP8, tag=f"msk{m}", name=f"msk{m}")
            for m in range(NM)
        ]

        # issue order: own+rag columns first (compute starts on them), masks
        # and valid on the gpsimd SWDGE queue (parallel issue lane), then the
        # far region in two big pieces per k-slice.
        head = OWNW + ragw
        far2 = (B - head) // 2
        pieces = [(0, head), (head, far2), (head + far2, B - head - far2)]
        for ks in range(KD):
            a, w = pieces[0]
            nc.sync.dma_start(
                et[:, ks:ks + 1, a:a + w], ET_d[ks * 128:(ks + 1) * 128, a:a + w]
            )
        for m in range(NM):
            nc.gpsimd.dma_start(msk[m][:], MSK_d[m * 128:(m + 1) * 128, :])
        nc.gpsimd.dma_start(val_t[:], val_d[:, :])
        for (a, w) in pieces[1:]:
            for ks in range(KD):
                nc.sync.dma_start(
                    et[:, ks:ks + 1, a:a + w], ET_d[ks * 128:(ks + 1) * 128, a:a + w]
                )

        # PE "touch" of each PE-read DMA region: a 1-element matmul waits on
        # the DMA semaphore so later matmuls need no cross-engine wait.
        def pe_touch(ap):
            t = psA.tile([128, 512], F32, tag="aux", name="touch")
            nc.tensor.matmul(t[0:1, 0:1], lhsT=ap, rhs=ap, start=True, stop=True)

        pe_touch(val_t[:, 0:1])
        for (a, w) in pieces:
            for ks in range(KD):
                pe_touch(et[:, ks:ks + 1, a:a + 1])
        # DVE touch of mask tiles (DVE reads them with psum ops later)
        dtch = smalls.tile([1, NM], F32, tag="dtch")
        for m in range(NM):
            nc.vector.tensor_copy(dtch[0:1, m:m + 1], msk[m][0:1, 0:1])
        # ACT warmup: trigger the activation-table load early (reads dtch).
        wact = smalls.tile([1, NM], FP16, tag="wact")
        nc.scalar.copy(wact[:], dtch[:])

        # ---- staging/accumulator tiles --------------------------------------
        # paired fp16 leaves [128, 1024]: P3 = [G | rag | ownh], P0..P2 = far
        # chunk pairs.  Wide tiles halve the DVE chain-op count.
        NPAIR = nfull // 2 + 1
        P = {}
        for pi in range(NPAIR):
            for m in range(NM):
                P[(pi, m)] = fstg.tile(
                    [128, 1024], FP16, tag=f"P{pi}m{m}", name=f"P{pi}m{m}"
                )
        acc = {}
        for lv in range(NPAIR - 1):
            for m in range(NM):
                acc[(lv, m)] = fstg.tile(
                    [128, 1024], FP16, tag=f"A{lv}m{m}", name=f"A{lv}m{m}"
                )

        mno4 = smalls.tile([128, NM], F32, tag="mno4")
        rtree4 = smalls.tile([128, NM], F32, tag="rtree4")
        mnh3 = smalls.tile([128, 1], F32, tag="mnh3")
        delta4 = smalls.tile([128, NM], F32, tag="delta4")
        rl4 = smalls.tile([128, NM], BF16, tag="rl4")
        pre4 = smalls.tile([128, NM], BF16, tag="pre4")
        ones_cf = const.tile([128, 1], BF16, tag="ones_cf")
        nc.vector.memset(ones_cf[:], 1.0)

        # ---- main loop (chunk-major) ----------------------------------------
        far_pos = 0
        for (kind, coff, cw) in order:
            pss = []
            for m in range(NM):
                ps = psM.tile([128, 512], F32, tag="ps", name="ps")
                a0 = W + m * 128
                for kk in range(KD // 2):
                    nc.tensor.matmul(
                        ps[:, :cw],
                        lhsT=et[:, 2 * kk:2 * kk + 2, a0:a0 + 128],
                        rhs=et[:, 2 * kk:2 * kk + 2, coff:coff + cw],
                        start=(kk == 0),
                        stop=(kk == KD // 2 - 1),
                        perf_mode=DR,
                    )
                pss.append(ps)

            if kind == "own":
                # masked own window -> P3 low half (DVE fuses mask add + copy)
                for m in range(NM):
                    nc.vector.tensor_tensor(
                        P[(3, m)][:, 0:512], pss[m][:], msk[m][:, 0:512], ALU.add
                    )
                # hardest-positive band: cols [128m, 128m+128+2W) of the window
                for m in range(NM):
                    lo = 128 * m
                    hi = min(128 * m + 128 + H2, 512)
                    nc.vector.tensor_reduce(
                        mno4[:, m:m + 1], P[(3, m)][:, lo:hi], AX.X, ALU.min
                    )
            elif kind == "ownh":
                # halo tail -> P3 filler slot.  Only tile m=3's band reaches
                # it (others have an all-zero mask there): m<3 plain ACT copy,
                # m=3 masked via DVE, plus its band-min part.
                for m in range(3):
                    nc.scalar.copy(P[(3, m)][:, 512 + ragw:512 + ragw + cw], pss[m][:, :cw])
                nc.vector.tensor_tensor(
                    P[(3, 3)][:, 512 + ragw:512 + ragw + cw], pss[3][:, :cw],
                    msk[3][:, 512:512 + cw], ALU.add
                )
                nc.vector.tensor_reduce(
                    mnh3[:], P[(3, 3)][:, 512 + ragw:512 + ragw + cw], AX.X, ALU.min
                )
            elif kind == "rag":
                # rag chunk -> P3 middle slot via ACT
                for m in range(NM):
                    nc.scalar.copy(P[(3, m)][:, 512:512 + cw], pss[m][:, :cw])
            else:
                fi = far_pos
                far_pos += 1
                pi, half = fi // 2, fi % 2
                for m in range(NM):
                    dst = P[(pi, m)][:, 512 * half:512 * half + cw]
                    if fi == 0:
                        # balance: DVE takes this copy slot (ACT is the
                        # busier reader)
                        nc.vector.tensor_copy(dst, pss[m][:, :cw])
                    else:
                        nc.scalar.copy(dst, pss[m][:, :cw])
                if half == 1:
                    # chain link once the pair is complete
                    for m in range(NM):
                        prev = P[(3, m)] if pi == 0 else acc[(pi - 1, m)]
                        nc.vector.tensor_tensor(
                            acc[(pi, m)][:], prev[:], P[(pi, m)][:], ALU.max
                        )

        # ---- final per-anchor math ------------------------------------------
        out_ps = psO.tile([1, NM], F32, tag="out_ps")
        for m in range(NM):
            nc.vector.tensor_reduce(
                rtree4[:, m:m + 1], acc[(NPAIR - 2, m)][:], AX.X, ALU.max
            )
        # hardest-positive: band min, plus the halo part for tile 3 only
        nc.vector.tensor_tensor(
            mno4[:, 3:4], mno4[:, 3:4], mnh3[:], ALU.min
        )
        nc.vector.tensor_tensor(delta4[:], rtree4[:], mno4[:], ALU.subtract)
        nc.scalar.activation(rl4[:], delta4[:], AF.Relu, bias=bconst[:])
        nc.vector.tensor_tensor(pre4[:], rl4[:], val_t[:], ALU.mult)
        nc.tensor.matmul(
            out_ps[:], lhsT=ones_cf[:], rhs=pre4[:], start=True, stop=True
        )
        out_sb = smalls.tile([1, NM], F32, tag="outsb")
        nc.vector.tensor_copy(out_sb[:], out_ps[:])
        nc.sync.dma_start(out_d[:, :], out_sb[:])

    nc.compile()
    return nc


def host_prepare(embeddings, labels):
    """Sort by label, normalize, fp8-quantize, build per-core layouts."""
    emb = np.asarray(embeddings, dtype=np.float32)
    labels = np.asarray(labels).astype(np.int64)
    order = np.argsort(labels, kind="stable")
    slab = labels[order]
    E = emb[order]
    nrm = np.maximum(np.linalg.norm(E, axis=1, keepdims=True), 1e-12)
    Q = (E / nrm).astype(ml_dtypes.float8_e4m3)
    ET = np.ascontiguousarray(Q.T)                 # [D, B] fp8
    ET2 = np.concatenate([ET, ET], axis=1)

    sizes = np.bincount(labels, minlength=C)
    msz = int(sizes.max())
    W = ((max(msz - 1, 1) + 15) // 16) * 16
    assert W <= 128, f"class span too large for this layout: {msz}"
    H2 = 2 * W
    OWNW = 512 + H2

    cnt = sizes[slab]
    valid_s = ((cnt >= 2) & (cnt <= B - 1)).astype(np.float32)

    in_maps = []
    for c in range(NCORES):
        start = (RPC * c - W) % B
        win = (start + np.arange(OWNW)) % B
        labs_win = slab[win]
        own_labs = slab[RPC * c:RPC * c + RPC]
        mask = np.where(
            own_labs[:, None] == labs_win[None, :], -4.0, 0.0
        ).astype(ml_dtypes.float8_e4m3)
        val = np.ascontiguousarray(
            valid_s[RPC * c:RPC * c + RPC].reshape(NM, 128).T
        ).astype(ml_dtypes.bfloat16)
        in_maps.append(
            {
                "ET": np.ascontiguousarray(ET2[:, start:start + B]),
                "MSK": mask,
                "valid": val,
            }
        )
    return in_maps, valid_s, W


_prog_cache = {}


def _get_program(W):
    key = (B, D, C, W)
    if key not in _prog_cache:
        _prog_cache[key] = build_program(W)
    return _prog_cache[key]


LAST_RESULT = None


def kernel(embeddings, labels):
    global LAST_RESULT
    in_maps, valid_s, W = host_prepare(embeddings, labels)
    nc = _get_program(W)
    trace = bool(int(os.environ.get("TRIPLET_TRACE", "0")))
    res = run_bass_kernel_spmd(nc, in_maps, list(range(NCORES)), trace=trace)
    LAST_RESULT = res
    loss_sum = float(sum(r["out"].astype(np.float64).sum() for r in res.results))
    n_valid = max(int(valid_s.sum()), 1)
    return np.array(loss_sum / n_valid, dtype=np.float32)


# revision 16
# speedup vs baseline: 1.2658x; 1.2658x over previous
"""BatchHardTripletLoss on 8 trn2 NeuronCores (Bass/Tile, SPMD data-parallel).

Label-sorted fp8 design:

Host stable-sorts rows by label, L2-normalizes, and quantizes to fp8 e4m3.
After sorting, all same-label pairs of an anchor lie within +-W columns of
its own position (W >= max class size - 1).  Each core c anchors sorted rows
[512c, 512c+512) and sees all 4096 columns in circular order starting at
512c - W, so its first 512+2W columns (the "own+halo" window) contain every
same-label column of its anchors.  Consequences:

  * the label mask (-4 on same-label pairs) only touches the own+halo
    window; it is applied on the Vector engine as a fused psum+mask ->
    fp16 copy (host ships a [512, 512+2W] fp8 additive mask), keeping the
    Tensor engine free for the Gram;
  * hardest-positive (min) only scans a 128+2W band of the masked window
    per 128-anchor tile;
  * the far chunks need only an unmasked max (hardest negative).

Main Gram runs as fp8 DoubleRow matmuls (K=256 per instruction; measured
~1.6x over plain fp8 on this part).  Max pipeline: ACT copies far psum
chunks -> fp16 SBUF leaves, DVE folds them into a running elementwise-max
chain, one final free-dim reduce per anchor tile.  The ragged far chunk
(480 cols) and the 2W halo tail share one 512-wide leaf, so every chain op
is a full [128, 512] tile and no psum-direct reductions trail the matmuls.
The valid-anchor mask is applied batched: relu losses [128, 4] bf16 are
premultiplied by the valid mask and partition-summed by one ones-matmul.

per-anchor loss = relu((max - min) + margin - 4); host adds the 8*4
partials and divides by n_valid.

Numerics: fp8 e4m3 Gram + fp16 staging measures rel err ~3e-4 vs the f32
reference on the fixed problem instance (tolerance 2e-2).
"""

import os
from contextlib import ExitStack

import numpy as np
import ml_dtypes

import concourse.bass as bass
import concourse.bacc as bacc
import concourse.mybir as mybir
import concourse.tile as tile
from concourse.bass_utils import run_bass_kernel_spmd

F32 = mybir.dt.float32
FP16 = mybir.dt.float16
BF16 = mybir.dt.bfloat16
FP8 = mybir.dt.float8e4
AF = mybir.ActivationFunctionType
ALU = mybir.AluOpType
AX = mybir.AxisListType
DR = mybir.MatmulPerfMode.DoubleRow

B, D, C = 4096, 512, 512
NCORES = 8
RPC = B // NCORES            # anchors per core = 512
NM = RPC // 128              # anchor tiles per core = 4
KD = D // 128                # k-subtiles = 4
MARGIN = 0.2
BIG = 4.0


def build_program(W):
    H2 = 2 * W
    OWNW = 512 + H2          # own+halo window width
    # column layout (host-chosen): [own+halo | ragged far | full far chunks]
    nfull = (B - OWNW) // 512
    ragw = B - OWNW - nfull * 512
    assert ragw + H2 == 512, (ragw, H2)   # rag + halo tail share one leaf
    # processing order: own512, halo tail, ragged far chunk, then full fars
    order = [("own", 0, 512), ("ownh", 512, H2), ("rag", OWNW, ragw)]
    order += [("far", OWNW + ragw + 512 * i, 512) for i in range(nfull)]
    NLEAF = nfull + 1        # leaf 0 = [rag | halo tail], leaves 1.. = far

    nc = bacc.Bacc("TRN2", target_bir_lowering=False, debug=False)
    ET_d = nc.declare_dram_parameter("ET", [D, B], FP8, isOutput=False)
    MSK_d = nc.declare_dram_parameter("MSK", [RPC, OWNW], FP8, isOutput=False)
    val_d = nc.declare_dram_parameter("valid", [128, NM], BF16, isOutput=False)
    out_d = nc.declare_dram_parameter("out", [1, NM], F32, isOutput=True)

    with tile.TileContext(nc) as tc, ExitStack() as ctx:
        const = ctx.enter_context(tc.tile_pool(name="const", bufs=1))
        bigp = ctx.enter_context(tc.tile_pool(name="bigp", bufs=1))
        fstg = ctx.enter_context(tc.tile_pool(name="fstg", bufs=1))
        smalls = ctx.enter_context(tc.tile_pool(name="small", bufs=1))
        psM = ctx.enter_context(tc.tile_pool(name="psM", bufs=6, space="PSUM"))
        psA = ctx.enter_context(tc.tile_pool(name="psA", bufs=1, space="PSUM"))
        psO = ctx.enter_context(tc.tile_pool(name="psO", bufs=1, space="PSUM"))

        bconst = const.tile([128, 1], F32, tag="bconst")
        nc.vector.memset(bconst[:], MARGIN - BIG)
        ones_cf = const.tile([128, 1], BF16, tag="ones_cf")
        nc.vector.memset(ones_cf[:], 1.0)

        # ---- input DMA ------------------------------------------------------
        val_t = const.tile([128, NM], BF16, tag="val")
        et = bigp.tile([128, KD, B], FP8, tag="et")
        msk = [
            smalls.tile([128, OWNW], FP8, tag=f"msk{m}", name=f"msk{m}")
            for m in range(NM)
        ]

        # issue order: own+rag columns first (compute starts on them), masks
        # and valid on the gpsimd SWDGE queue (parallel issue lane), then the
        # far region in two big pieces per k-slice.
        head = OWNW + ragw
        far2 = (B - head) // 2
        pieces = [(0, head), (head, far2), (head + far2, B - head - far2)]
        for ks in range(KD):
            a, w = pieces[0]
            nc.sync.dma_start(
                et[:, ks:ks + 1, a:a + w], ET_d[ks * 128:(ks + 1) * 128, a:a + w]
            )
        for m in range(NM):
            nc.gpsimd.dma_start(msk[m][:], MSK_d[m * 128:(m + 1) * 128, :])
        nc.gpsimd.dma_start(val_t[:], val_d[:, :])
        for (a, w) in pieces[1:]:
            for ks in range(KD):
                nc.sync.dma_start(
                    et[:, ks:ks + 1, a:a + w], ET_d[ks * 128:(ks + 1) * 128, a:a + w]
                )

        # PE "touch" of each PE-read DMA region: a 1-element matmul waits on
        # the DMA semaphore so later matmuls need no cross-engine wait.
        def pe_touch(ap):
            t = psA.tile([128, 512], F32, tag="aux", name="touch")
            nc.tensor.matmul(t[0:1, 0:1], lhsT=ap, rhs=ap, start=True, stop=True)

        pe_touch(val_t[:, 0:1])
        for (a, w) in pieces:
            for ks in range(KD):
                pe_touch(et[:, ks:ks + 1, a:a + 1])
        # DVE touch of mask tiles (DVE reads them with psum ops later)
        dtch = smalls.tile([1, NM], F32, tag="dtch")
        for m in range(NM):
            nc.vector.tensor_copy(dtch[0:1, m:m + 1], msk[m][0:1, 0:1])
        # ACT warmup: trigger the activation-table load early (reads dtch).
        wact = smalls.tile([1, NM], FP16, tag="wact")
        nc.scalar.copy(wact[:], dtch[:])

        # ---- staging/accumulator tiles --------------------------------------
        G = [fstg.tile([128, 512], FP16, tag=f"G{m}", name=f"G{m}")
             for m in range(NM)]
        F = {}
        for fi in range(NLEAF):
            for m in range(NM):
                F[(fi, m)] = fstg.tile(
                    [128, 512], FP16, tag=f"F{fi}m{m}", name=f"F{fi}m{m}"
                )
        acc = {}
        for lv in range(NLEAF):
            for m in range(NM):
                acc[(lv, m)] = fstg.tile(
                    [128, 512], FP16, tag=f"A{lv}m{m}", name=f"A{lv}m{m}"
                )

        mno4 = smalls.tile([128, NM], F32, tag="mno4")
        rtree4 = smalls.tile([128, NM], F32, tag="rtree4")
        mnh3 = smalls.tile([128, 1], F32, tag="mnh3")
        delta4 = smalls.tile([128, NM], F32, tag="delta4")
        rl4 = smalls.tile([128, NM], BF16, tag="rl4")
        pre4 = smalls.tile([128, NM], BF16, tag="pre4")

        # ---- main loop (chunk-major) ----------------------------------------
        far_pos = 1
        for (kind, coff, cw) in order:
            pss = []
            for m in range(NM):
                ps = psM.tile([128, 512], F32, tag="ps", name="ps")
                a0 = W + m * 128
                for kk in range(KD // 2):
                    nc.tensor.matmul(
                        ps[:, :cw],
                        lhsT=et[:, 2 * kk:2 * kk + 2, a0:a0 + 128],
                        rhs=et[:, 2 * kk:2 * kk + 2, coff:coff + cw],
                        start=(kk == 0),
                        stop=(kk == KD // 2 - 1),
                        perf_mode=DR,
                    )
                pss.append(ps)

            if kind == "own":
                # masked own window -> G (DVE fuses mask add + fp16 copy)
                for m in range(NM):
                    nc.vector.tensor_tensor(
                        G[m][:], pss[m][:], msk[m][:, 0:512], ALU.add
                    )
                # hardest-positive band: cols [128m, 128m+128+2W) of the window
                for m in range(NM):
                    lo = 128 * m
                    hi = min(128 * m + 128 + H2, 512)
                    nc.vector.tensor_reduce(
                        mno4[:, m:m + 1], G[m][:, lo:hi], AX.X, ALU.min
                    )
            elif kind == "ownh":
                # halo tail -> tail slot of leaf 0.  Only tile m=3's band
                # reaches it (others have an all-zero mask there): m<3 plain
                # ACT copy, m=3 masked via DVE plus its band-min part.
                for m in range(3):
                    nc.scalar.copy(F[(0, m)][:, ragw:ragw + cw], pss[m][:, :cw])
                nc.vector.tensor_tensor(
                    F[(0, 3)][:, ragw:ragw + cw], pss[3][:, :cw],
                    msk[3][:, 512:512 + cw], ALU.add
                )
                nc.vector.tensor_reduce(
                    mnh3[:], F[(0, 3)][:, ragw:ragw + cw], AX.X, ALU.min
                )
            elif kind == "rag":
                for m in range(NM):
                    nc.scalar.copy(F[(0, m)][:, :cw], pss[m][:, :cw])
                # leaf 0 complete -> first chain link
                for m in range(NM):
                    nc.vector.tensor_tensor(
                        acc[(0, m)][:], G[m][:], F[(0, m)][:], ALU.max
                    )
            else:
                fi = far_pos
                far_pos += 1
                for m in range(NM):
                    if fi == 1:
                        # balance: DVE takes this copy slot (ACT is busier)
                        nc.vector.tensor_copy(F[(fi, m)][:], pss[m][:])
                    else:
                        nc.scalar.copy(F[(fi, m)][:], pss[m][:])
                for m in range(NM):
                    nc.vector.tensor_tensor(
                        acc[(fi, m)][:], acc[(fi - 1, m)][:], F[(fi, m)][:],
                        ALU.max
                    )

        # ---- final per-anchor math (batched over the 4 anchor tiles) --------
        out_ps = psO.tile([1, NM], F32, tag="out_ps")
        for m in range(NM):
            nc.vector.tensor_reduce(
                rtree4[:, m:m + 1], acc[(NLEAF - 1, m)][:], AX.X, ALU.max
            )
        nc.vector.tensor_tensor(mno4[:, 3:4], mno4[:, 3:4], mnh3[:], ALU.min)
        nc.vector.tensor_tensor(delta4[:], rtree4[:], mno4[:], ALU.subtract)
        nc.scalar.activation(rl4[:], delta4[:], AF.Relu, bias=bconst[:])
        nc.vector.tensor_tensor(pre4[:], rl4[:], val_t[:], ALU.mult)
        nc.tensor.matmul(
            out_ps[:], lhsT=ones_cf[:], rhs=pre4[:], start=True, stop=True
        )
        out_sb = smalls.tile([1, NM], F32, tag="outsb")
        nc.vector.tensor_copy(out_sb[:], out_ps[:])
        nc.sync.dma_start(out_d[:, :], out_sb[:])

    nc.compile()
    return nc


def host_prepare(embeddings, labels):
    """Sort by label, normalize, fp8-quantize, build per-core layouts."""
    emb = np.asarray(embeddings, dtype=np.float32)
    labels = np.asarray(labels).astype(np.int64)
    order = np.argsort(labels, kind="stable")
    slab = labels[order]
    E = emb[order]
    nrm = np.maximum(np.linalg.norm(E, axis=1, keepdims=True), 1e-12)
    Q = (E / nrm).astype(ml_dtypes.float8_e4m3)
    ET = np.ascontiguousarray(Q.T)                 # [D, B] fp8
    ET2 = np.concatenate([ET, ET], axis=1)

    sizes = np.bincount(labels, minlength=C)
    msz = int(sizes.max())
    W = ((max(msz - 1, 1) + 15) // 16) * 16
    assert W <= 128, f"class span too large for this layout: {msz}"
    H2 = 2 * W
    OWNW = 512 + H2

    cnt = sizes[slab]
    valid_s = ((cnt >= 2) & (cnt <= B - 1)).astype(np.float32)

    in_maps = []
    for c in range(NCORES):
        start = (RPC * c - W) % B
        win = (start + np.arange(OWNW)) % B
        labs_win = slab[win]
        own_labs = slab[RPC * c:RPC * c + RPC]
        mask = np.where(
            own_labs[:, None] == labs_win[None, :], -4.0, 0.0
        ).astype(ml_dtypes.float8_e4m3)
        val = np.ascontiguousarray(
            valid_s[RPC * c:RPC * c + RPC].reshape(NM, 128).T
        ).astype(ml_dtypes.bfloat16)
        in_maps.append(
            {
                "ET": np.ascontiguousarray(ET2[:, start:start + B]),
                "MSK": mask,
                "valid": val,
            }
        )
    return in_maps, valid_s, W


_prog_cache = {}


def _get_program(W):
    key = (B, D, C, W)
    if key not in _prog_cache:
        _prog_cache[key] = build_program(W)
    return _prog_cache[key]


LAST_RESULT = None


def kernel(embeddings, labels):
    global LAST_RESULT
    in_maps, valid_s, W = host_prepare(embeddings, labels)
    nc = _get_program(W)
    trace = bool(int(os.environ.get("TRIPLET_TRACE", "0")))
    res = run_bass_kernel_spmd(nc, in_maps, list(range(NCORES)), trace=trace)
    LAST_RESULT = res
    loss_sum = float(sum(r["out"].astype(np.float64).sum() for r in res.results))
    n_valid = max(int(valid_s.sum()), 1)
    return np.array(loss_sum / n_valid, dtype=np.float32)


# revision 17
# speedup vs baseline: 1.2952x; 1.0232x over previous
"""BatchHardTripletLoss on 8 trn2 NeuronCores (Bass/Tile, SPMD data-parallel).

Label-sorted fp8 design:

Host stable-sorts rows by label, L2-normalizes, and quantizes to fp8 e4m3.
After sorting, all same-label pairs of an anchor lie within +-W columns of
its own position (W >= max class size - 1).  Each core c anchors sorted rows
[512c, 512c+512) and sees all 4096 columns in circular order starting at
512c - W, so its first 512+2W columns (the "own+halo" window) contain every
same-label column of its anchors.  Consequences:

  * the label mask (-4 on same-label pairs) only touches the own+halo
    window; it is applied on the Vector engine as a fused psum+mask ->
    fp16 copy (host ships a [512, 512+2W] fp8 additive mask), keeping the
    Tensor engine free for the Gram;
  * hardest-positive (min) only scans a 128+2W band of the masked window
    per 128-anchor tile;
  * the far chunks need only an unmasked max (hardest negative).

Main Gram runs as fp8 DoubleRow matmuls (K=256 per instruction; measured
~1.6x over plain fp8 on this part).  Max pipeline: ACT copies far psum
chunks -> fp16 SBUF leaves, DVE folds them into a running elementwise-max
chain, one final free-dim reduce per anchor tile.  The ragged far chunk
(480 cols) and the 2W halo tail share one 512-wide leaf, so every chain op
is a full [128, 512] tile and no psum-direct reductions trail the matmuls.
The valid-anchor mask is applied batched: relu losses [128, 4] bf16 are
premultiplied by the valid mask and partition-summed by one ones-matmul.

per-anchor loss = relu((max - min) + margin - 4); host adds the 8*4
partials and divides by n_valid.

Numerics: fp8 e4m3 Gram + fp16 staging measures rel err ~3e-4 vs the f32
reference on the fixed problem instance (tolerance 2e-2).
"""

import os
from contextlib import ExitStack

import numpy as np
import ml_dtypes

import concourse.bass as bass
import concourse.bacc as bacc
import concourse.mybir as mybir
import concourse.tile as tile
from concourse.bass_utils import run_bass_kernel_spmd

F32 = mybir.dt.float32
FP16 = mybir.dt.float16
BF16 = mybir.dt.bfloat16
FP8 = mybir.dt.float8e4
AF = mybir.ActivationFunctionType
ALU = mybir.AluOpType
AX = mybir.AxisListType
DR = mybir.MatmulPerfMode.DoubleRow

B, D, C = 4096, 512, 512
NCORES = 8
RPC = B // NCORES            # anchors per core = 512
NM = RPC // 128              # anchor tiles per core = 4
KD = D // 128                # k-subtiles = 4
MARGIN = 0.2
BIG = 4.0


def build_program(W):
    H2 = 2 * W
    OWNW = 512 + H2          # own+halo window width
    # column layout (host-chosen): [own+halo | ragged far | full far chunks]
    nfull = (B - OWNW) // 512
    ragw = B - OWNW - nfull * 512
    assert ragw + H2 == 512, (ragw, H2)   # halo tail + rag share one chunk
    # processing order: own512, tail (= halo tail + ragged far), full fars
    order = [("own", 0, 512), ("tail", 512, 512)]
    order += [("far", 1024 + 512 * i, 512) for i in range(nfull)]
    NLEAF = nfull + 1        # leaf 0 = [halo tail | rag], leaves 1.. = far

    nc = bacc.Bacc("TRN2", target_bir_lowering=False, debug=False)
    ET_d = nc.declare_dram_parameter("ET", [D, B], FP8, isOutput=False)
    MSK_d = nc.declare_dram_parameter("MSK", [RPC, OWNW], FP8, isOutput=False)
    val_d = nc.declare_dram_parameter("valid", [128, NM], BF16, isOutput=False)
    out_d = nc.declare_dram_parameter("out", [1, NM], F32, isOutput=True)

    with tile.TileContext(nc) as tc, ExitStack() as ctx:
        const = ctx.enter_context(tc.tile_pool(name="const", bufs=1))
        bigp = ctx.enter_context(tc.tile_pool(name="bigp", bufs=1))
        fstg = ctx.enter_context(tc.tile_pool(name="fstg", bufs=1))
        smalls = ctx.enter_context(tc.tile_pool(name="small", bufs=1))
        psM = ctx.enter_context(tc.tile_pool(name="psM", bufs=6, space="PSUM"))
        psA = ctx.enter_context(tc.tile_pool(name="psA", bufs=1, space="PSUM"))
        psO = ctx.enter_context(tc.tile_pool(name="psO", bufs=1, space="PSUM"))

        bconst = const.tile([128, 1], F32, tag="bconst")
        nc.vector.memset(bconst[:], MARGIN - BIG)
        ones_cf = const.tile([128, 1], BF16, tag="ones_cf")
        nc.vector.memset(ones_cf[:], 1.0)

        # ---- input DMA ------------------------------------------------------
        val_t = const.tile([128, NM], BF16, tag="val")
        et = bigp.tile([128, KD, B], FP8, tag="et")
        msk = [
            smalls.tile([128, OWNW], FP8, tag=f"msk{m}", name=f"msk{m}")
            for m in range(NM)
        ]

        # issue order: own+rag columns first (compute starts on them), masks
        # and valid on the gpsimd SWDGE queue (parallel issue lane), then the
        # far region in two big pieces per k-slice.
        head = OWNW + ragw
        far2 = (B - head) // 2
        pieces = [(0, head), (head, far2), (head + far2, B - head - far2)]
        for ks in range(KD):
            a, w = pieces[0]
            nc.sync.dma_start(
                et[:, ks:ks + 1, a:a + w], ET_d[ks * 128:(ks + 1) * 128, a:a + w]
            )
        for m in range(NM):
            nc.gpsimd.dma_start(msk[m][:], MSK_d[m * 128:(m + 1) * 128, :])
        nc.gpsimd.dma_start(val_t[:], val_d[:, :])
        for (a, w) in pieces[1:]:
            for ks in range(KD):
                nc.sync.dma_start(
                    et[:, ks:ks + 1, a:a + w], ET_d[ks * 128:(ks + 1) * 128, a:a + w]
                )

        # PE "touch" of each PE-read DMA region: a 1-element matmul waits on
        # the DMA semaphore so later matmuls need no cross-engine wait.
        def pe_touch(ap):
            t = psA.tile([128, 512], F32, tag="aux", name="touch")
            nc.tensor.matmul(t[0:1, 0:1], lhsT=ap, rhs=ap, start=True, stop=True)

        pe_touch(val_t[:, 0:1])
        for ks in range(KD):
            pe_touch(et[:, ks:ks + 1, 0:1])
        # DVE touch of mask tiles (DVE reads them with psum ops later)
        dtch = smalls.tile([1, NM], F32, tag="dtch")
        for m in range(NM):
            nc.vector.tensor_copy(dtch[0:1, m:m + 1], msk[m][0:1, 0:1])
        # ACT warmup: trigger the activation-table load early (reads dtch).
        wact = smalls.tile([1, NM], FP16, tag="wact")
        nc.scalar.copy(wact[:], dtch[:])

        # ---- staging/accumulator tiles --------------------------------------
        G = [fstg.tile([128, 512], FP16, tag=f"G{m}", name=f"G{m}")
             for m in range(NM)]
        F = {}
        for fi in range(NLEAF):
            for m in range(NM):
                F[(fi, m)] = fstg.tile(
                    [128, 512], FP16, tag=f"F{fi}m{m}", name=f"F{fi}m{m}"
                )
        acc = {}
        for lv in range(NLEAF):
            for m in range(NM):
                acc[(lv, m)] = fstg.tile(
                    [128, 512], FP16, tag=f"A{lv}m{m}", name=f"A{lv}m{m}"
                )

        mno4 = smalls.tile([128, NM], F32, tag="mno4")
        rtree4 = smalls.tile([128, NM], F32, tag="rtree4")
        mnh3 = smalls.tile([128, 1], F32, tag="mnh3")
        delta4 = smalls.tile([128, NM], F32, tag="delta4")
        rl4 = smalls.tile([128, NM], BF16, tag="rl4")
        pre4 = smalls.tile([128, NM], BF16, tag="pre4")

        # ---- main loop (chunk-major) ----------------------------------------
        # far-piece touches are interleaved: emitted just before the first
        # chunk that reads the piece, so early matmuls don't wait on far DMAs.
        touch_at = {1024: 1, 1024 + 3 * 512: 2}
        far_pos = 1
        for (kind, coff, cw) in order:
            if coff in touch_at:
                a, w = pieces[touch_at[coff]]
                for ks in range(KD):
                    pe_touch(et[:, ks:ks + 1, a:a + 1])
            pss = []
            for m in range(NM):
                ps = psM.tile([128, 512], F32, tag="ps", name="ps")
                a0 = W + m * 128
                for kk in range(KD // 2):
                    nc.tensor.matmul(
                        ps[:, :cw],
                        lhsT=et[:, 2 * kk:2 * kk + 2, a0:a0 + 128],
                        rhs=et[:, 2 * kk:2 * kk + 2, coff:coff + cw],
                        start=(kk == 0),
                        stop=(kk == KD // 2 - 1),
                        perf_mode=DR,
                    )
                pss.append(ps)

            if kind == "own":
                # masked own window -> G (DVE fuses mask add + fp16 copy)
                for m in range(NM):
                    nc.vector.tensor_tensor(
                        G[m][:], pss[m][:], msk[m][:, 0:512], ALU.add
                    )
                # hardest-positive band: cols [128m, 128m+128+2W) of the window
                for m in range(NM):
                    lo = 128 * m
                    hi = min(128 * m + 128 + H2, 512)
                    nc.vector.tensor_reduce(
                        mno4[:, m:m + 1], G[m][:, lo:hi], AX.X, ALU.min
                    )
            elif kind == "tail":
                # [halo tail | rag] -> leaf 0.  Only tile m=3's band reaches
                # the halo columns (others have an all-zero mask there):
                # m<3 plain ACT copy, m=3 masked halo part via DVE.
                for m in range(3):
                    nc.scalar.copy(F[(0, m)][:], pss[m][:])
                nc.vector.tensor_tensor(
                    F[(0, 3)][:, 0:H2], pss[3][:, 0:H2],
                    msk[3][:, 512:512 + H2], ALU.add
                )
                nc.scalar.copy(F[(0, 3)][:, H2:512], pss[3][:, H2:512])
                nc.vector.tensor_reduce(
                    mnh3[:], F[(0, 3)][:, 0:H2], AX.X, ALU.min
                )
                # leaf 0 complete -> first chain link
                for m in range(NM):
                    nc.vector.tensor_tensor(
                        acc[(0, m)][:], G[m][:], F[(0, m)][:], ALU.max
                    )
            else:
                fi = far_pos
                far_pos += 1
                for m in range(NM):
                    if fi == 1:
                        # balance: DVE takes this copy slot (ACT is busier)
                        nc.vector.tensor_copy(F[(fi, m)][:], pss[m][:])
                    else:
                        nc.scalar.copy(F[(fi, m)][:], pss[m][:])
                for m in range(NM):
                    nc.vector.tensor_tensor(
                        acc[(fi, m)][:], acc[(fi - 1, m)][:], F[(fi, m)][:],
                        ALU.max
                    )

        # ---- final per-anchor math (batched over the 4 anchor tiles) --------
        out_ps = psO.tile([1, NM], F32, tag="out_ps")
        for m in range(NM):
            nc.vector.tensor_reduce(
                rtree4[:, m:m + 1], acc[(NLEAF - 1, m)][:], AX.X, ALU.max
            )
        nc.vector.tensor_tensor(mno4[:, 3:4], mno4[:, 3:4], mnh3[:], ALU.min)
        nc.vector.tensor_tensor(delta4[:], rtree4[:], mno4[:], ALU.subtract)
        nc.scalar.activation(rl4[:], delta4[:], AF.Relu, bias=bconst[:])
        nc.vector.tensor_tensor(pre4[:], rl4[:], val_t[:], ALU.mult)
        nc.tensor.matmul(
            out_ps[:], lhsT=ones_cf[:], rhs=pre4[:], start=True, stop=True
        )
        out_sb = smalls.tile([1, NM], F32, tag="outsb")
        nc.vector.tensor_copy(out_sb[:], out_ps[:])
        nc.sync.dma_start(out_d[:, :], out_sb[:])

    nc.compile()
    return nc


def host_prepare(embeddings, labels):
    """Sort by label, normalize, fp8-quantize, build per-core layouts."""
    emb = np.asarray(embeddings, dtype=np.float32)
    labels = np.asarray(labels).astype(np.int64)
    order = np.argsort(labels, kind="stable")
    slab = labels[order]
    E = emb[order]
    nrm = np.maximum(np.linalg.norm(E, axis=1, keepdims=True), 1e-12)
    Q = (E / nrm).astype(ml_dtypes.float8_e4m3)
    ET = np.ascontiguousarray(Q.T)                 # [D, B] fp8
    ET2 = np.concatenate([ET, ET], axis=1)

    sizes = np.bincount(labels, minlength=C)
    msz = int(sizes.max())
    W = ((max(msz - 1, 1) + 15) // 16) * 16
    assert W <= 128, f"class span too large for this layout: {msz}"
    H2 = 2 * W
    OWNW = 512 + H2

    cnt = sizes[slab]
    valid_s = ((cnt >= 2) & (cnt <= B - 1)).astype(np.float32)

    in_maps = []
    for c in range(NCORES):
        start = (RPC * c - W) % B
        win = (start + np.arange(OWNW)) % B
        labs_win = slab[win]
        own_labs = slab[RPC * c:RPC * c + RPC]
        mask = np.where(
            own_labs[:, None] == labs_win[None, :], -4.0, 0.0
        ).astype(ml_dtypes.float8_e4m3)
        val = np.ascontiguousarray(
            valid_s[RPC * c:RPC * c + RPC].reshape(NM, 128).T
        ).astype(ml_dtypes.bfloat16)
        in_maps.append(
            {
                "ET": np.ascontiguousarray(ET2[:, start:start + B]),
                "MSK": mask,
                "valid": val,
            }
        )
    return in_maps, valid_s, W


_prog_cache = {}


def _get_program(W):
    key = (B, D, C, W)
    if key not in _prog_cache:
        _prog_cache[key] = build_program(W)
    return _prog_cache[key]


LAST_RESULT = None


def kernel(embeddings, labels):
    global LAST_RESULT
    in_maps, valid_s, W = host_prepare(embeddings, labels)
    nc = _get_program(W)
    trace = bool(int(os.environ.get("TRIPLET_TRACE", "0")))
    res = run_bass_kernel_spmd(nc, in_maps, list(range(NCORES)), trace=trace)
    LAST_RESULT = res
    loss_sum = float(sum(r["out"].astype(np.float64).sum() for r in res.results))
    n_valid = max(int(valid_s.sum()), 1)
    return np.array(loss_sum / n_valid, dtype=np.float32)
